# revision 16
# baseline (speedup 1.0000x reference)
# Trainium2 Bass kernel for ClassAttn (single class-token query attention).
#
# Math (per batch b):
#   q   = x[b,0] @ Wq * scale                       [CR]
#   logits[h,n] = sum_c x[b,n,c] * wq_eff[c,h]      with wq_eff[c,h] = sum_d Wk[c,h*HD+d] q[h*HD+d]
#   w = exp(logits)          (softmax needs no max-subtraction; inputs bounded)
#   z[h] = sum_n w[h,n]
#   s[h,c] = sum_n w[h,n] x[b,n,c]
#   o[h,d] = (1/z[h]) sum_c s[h,c] Wv[c,h*HD+d]
#   out = o.flatten() @ Wp + bp
#
# Sharding: data-parallel over batch, 8 cores x 4 batches, no collectives.
# Per-core the kernel is DMA-paced (64 MiB of x at ~360 GB/s ~ 187 us); the
# PE work per 512-token group is kept under the DMA time:
#   - x transposes: 8 psum quads (4x128x128 each), drains alternate DVE/scalar
#     and cast bf16->fp8 producing xt8.
#   - logits: 4 fp8e4 DoubleRow pairs (2 c-blocks per pass, 0.5 cyc/row);
#     wq_eff is prescaled x32 into fp8-normal range, exp applies scale 1/32.
#   - wq_eff itself is computed on the PE from a host-side transposed Wk
#     (WkT) and a head-masked q-hat (mask folds in SCALE*32), not on DVE.
#   - s-accum stays bf16 from the natural-layout xg.
#   - epilogue (s/z, Wv, Wp, bias) is batched across all 4 batches at the end.

import numpy as np
from contextlib import ExitStack

import concourse.bass as bass
import concourse.mybir as mybir
import concourse.tile as tile
from concourse import bacc
from concourse.masks import make_identity

F32 = mybir.dt.float32
BF16 = mybir.dt.bfloat16
FP8 = mybir.dt.float8e4

B, N, C = 32, 4096, 1024
H, HD = 16, 16
CR = 256
SCALE = HD ** -0.5
NCORES = 8
BS = B // NCORES          # batches per core
GTOK = 512                # tokens per group
BLK = 128                 # tokens per block (partition tile)
NBLK = GTOK // BLK        # 4 blocks per group
NCB = C // 128            # 8 c-blocks
WQ_PRESCALE = 1.0        # wq_eff kept x32 in fp8; exp() applies 1/32


def emit(tc, x_d, wq_d, wkt_d, wv_d, wp_d, bp_d, hmask_d, dmask_d, out_d, bs, n):
    nc = tc.nc
    ngroups = n // GTOK
    nslots = bs * ngroups
    with ExitStack() as ctx:
        const = ctx.enter_context(tc.tile_pool(name="const", bufs=1))
        px = ctx.enter_context(tc.tile_pool(name="px", bufs=5))
        pxt = ctx.enter_context(tc.tile_pool(name="pxt", bufs=3))
        pw = ctx.enter_context(tc.tile_pool(name="pw", bufs=3))
        pb2 = ctx.enter_context(tc.tile_pool(name="pb2", bufs=2))
        pb4 = ctx.enter_context(tc.tile_pool(name="pb4", bufs=4))
        ps_xt = ctx.enter_context(tc.tile_pool(name="ps_xt", bufs=4, space="PSUM"))
        ps_lg = ctx.enter_context(tc.tile_pool(name="ps_lg", bufs=1, space="PSUM"))
        ps_sm = ctx.enter_context(tc.tile_pool(name="ps_sm", bufs=1, space="PSUM"))
        ps_s = ctx.enter_context(tc.tile_pool(name="ps_s", bufs=1, space="PSUM"))

        # ---- constants / weights ----
        ident = const.tile([128, 128], BF16)
        make_identity(nc, ident[:])
        s_inv = const.tile([16, 1], F32)
        nc.vector.memset(s_inv[:], 1.0 / WQ_PRESCALE)

        wq_sb = const.tile([128, NCB, CR], BF16)     # Wq[c,r] c-blocked
        nc.gpsimd.dma_start(out=wq_sb[:], in_=wq_d.rearrange("(p j) r -> p j r", p=128))
        wkt_sb = const.tile([128, 2, C], BF16)       # WkT[r,c] r-blocked
        nc.gpsimd.dma_start(out=wkt_sb[:], in_=wkt_d.rearrange("(j p) c -> p j c", p=128))
        wv_sb = const.tile([128, NCB, CR], BF16)     # Wv[c,r]
        nc.gpsimd.dma_start(out=wv_sb[:], in_=wv_d.rearrange("(j p) r -> p j r", p=128))
        wp_sb = const.tile([128, 2, C], BF16)        # Wp[r,c] r-blocked
        nc.gpsimd.dma_start(out=wp_sb[:], in_=wp_d.rearrange("(j p) c -> p j c", p=128))
        bp_sb = const.tile([bs, C], F32)
        nc.sync.dma_start(out=bp_sb[:], in_=bp_d)
        hmask_sb = const.tile([128, 2, H], F32)      # SCALE*32 at (r, head(r))
        nc.sync.dma_start(out=hmask_sb[:], in_=hmask_d)
        dmask_sb = const.tile([128, 2, bs, H], F32)  # head-extract mask, b-replicated
        nc.sync.dma_start(out=dmask_sb[:], in_=dmask_d)

        xgs = {}      # slot -> xg tile
        xts = {}      # slot -> xt8 tile
        wTs = {}      # slot -> wT tile (exp output)
        wsbs = {}     # slot -> w_sb tile (w natural)
        xqs = {}      # batch -> xq tile
        qhs = {}      # batch -> qh_bf tile
        wq8s = {}     # batch -> wq8 tile
        zgs = {}      # batch -> zg tile
        ssbs = {}     # batch -> s_sb tile
        sps = {}      # batch -> s psum tile

        def emit_xq(b):
            """Class-token channels, c on partitions (tiny strided cast DMA)."""
            xq = px.tile([128, NCB], BF16, tag="xq")
            nc.gpsimd.dma_start(out=xq[:], in_=x_d[b, 0, :].rearrange("(p j) -> p j", p=128))
            xqs[b] = xq

        def emit_q(b):
            """qT[r] = sum_c Wq[c,r] x0[c]; 16 tiny matmuls, rows=1."""
            xq = xqs[b]
            q_ps = ps_sm.tile([128, 2], F32, tag="sm")
            for rh in range(2):
                for j in range(NCB):
                    nc.tensor.matmul(
                        q_ps[:, rh : rh + 1],
                        wq_sb[:, j, rh * 128 : (rh + 1) * 128],
                        xq[:, j : j + 1],
                        start=(j == 0), stop=(j == NCB - 1),
                    )
            q_sb = pb2.tile([128, 2], F32, tag="q_sb")
            nc.vector.tensor_copy(q_sb[:], q_ps[:])
            qh = pb2.tile([128, 2, H], BF16, tag="qh")
            for rh in range(2):
                nc.vector.tensor_scalar_mul(
                    qh[:, rh, :], hmask_sb[:, rh, :], q_sb[:, rh : rh + 1]
                )
            qhs[b] = qh

        def emit_wqeff(b):
            """wq8[c,j,h] = 32*SCALE * sum_d Wk[c,(h,d)] q[(h,d)], fp8."""
            qh = qhs[b]
            wq8 = pb2.tile([128, NCB, H], BF16, tag="wq8")
            for j in range(NCB):
                we_ps = ps_sm.tile([128, H], F32, tag="sm")
                for rh in range(2):
                    nc.tensor.matmul(
                        we_ps[:],
                        wkt_sb[:, rh, j * 128 : (j + 1) * 128],
                        qh[:, rh, :],
                        start=(rh == 0), stop=(rh == 1),
                    )
                nc.vector.tensor_copy(wq8[:, j, :], we_ps[:])
            wq8s[b] = wq8

        def emit_dma(k):
            b, g = divmod(k, ngroups)
            xg = px.tile([128, NBLK, C], BF16, tag="xg")
            # Token n' = t*128+p of this group holds DRAM token g*GTOK + 4p + t:
            # 16 KB contiguous DRAM per partition descriptor.
            nc.gpsimd.dma_start(
                out=xg[:],
                in_=x_d[b, g * GTOK : (g + 1) * GTOK, :].rearrange(
                    "(p t) c -> p t c", t=NBLK
                ),
            )
            xgs[k] = xg

        def emit_T_quad(k, quad):
            """4 PE transposes into one psum bank + one casting drain -> xt8."""
            xg = xgs[k]
            if quad == 0:
                xts[k] = pxt.tile([128, NCB, GTOK], BF16, tag="xt", name="xt8")
            xt8 = xts[k]
            blk, jh = divmod(quad, 2)
            xt_ps = ps_xt.tile([128, 4, 128], BF16, tag="xt_ps")
            for jj in range(4):
                j = jh * 4 + jj
                nc.tensor.transpose(
                    xt_ps[:, jj, :], xg[:, blk, j * 128 : (j + 1) * 128], ident[:]
                )
            dst = xt8[:, jh * 4 : (jh + 1) * 4, blk * BLK : (blk + 1) * BLK]
            nc.vector.tensor_copy(dst, xt_ps[:])

        def emit_C2_pair(k, idx):
            """2 of the 8 s-accum matmuls for slot k (bf16, natural xg)."""
            b, g = divmod(k, ngroups)
            xg = xgs[k]
            w_sb = wsbs[k]
            s_ps = sps[b]
            for u in range(2):
                blk, half = divmod(2 * idx + u, 2)
                first = g == 0 and blk == 0
                last = g == ngroups - 1 and blk == NBLK - 1
                nc.tensor.matmul(
                    s_ps[:, half * 512 : (half + 1) * 512],
                    w_sb[:, blk, :],
                    xg[:, blk, half * 512 : (half + 1) * 512],
                    start=first, stop=last,
                )

        def emit_C1(k):
            """logits -> exp(x/32) with z-accum; 4 fp8 DoubleRow pairs."""
            b, g = divmod(k, ngroups)
            xt8 = xts[k]
            wq8 = wq8s[b]
            if g == 0:
                zgs[b] = pb4.tile([16, ngroups], F32, tag="zg", name="zg")
            lg_ps = ps_lg.tile([16, GTOK], F32, tag="lg")
            for p in range(NCB):
                nc.tensor.matmul(
                    lg_ps[:], wq8[:, p, :], xt8[:, p, :],
                    start=(p == 0), stop=(p == NCB - 1),
                )
            wT = pw.tile([16, GTOK], BF16, tag="wT")
            nc.scalar.activation(
                wT[:], lg_ps[:], mybir.ActivationFunctionType.Exp,
                accum_out=zgs[b][:, g : g + 1],
            )
            wTs[k] = wT

        def emit_wtr(k):
            """w natural layout via 4 mini PE transposes."""
            wT = wTs.pop(k)
            w_sb = pw.tile([128, NBLK, H], BF16, tag="w_sb")
            w_ps = ps_sm.tile([128, NBLK, H], BF16, tag="sm")
            for blk in range(NBLK):
                nc.tensor.transpose(
                    w_ps[:, blk, :], wT[:, blk * BLK : (blk + 1) * BLK], ident[:16, :16]
                )
            nc.scalar.copy(w_sb[:], w_ps[:])
            wsbs[k] = w_sb

        def emit_sdrain(b):
            """Drain the finished s psum for batch b to SBUF."""
            s_sb = pb4.tile([16, C], F32, tag="s_sb")
            nc.scalar.copy(s_sb[:], sps.pop(b)[:])
            ssbs[b] = s_sb

        def emit_E():
            """Batched epilogue: s/z -> Wv (block-diag) -> Wp + bias, all b."""
            sbar = pb2.tile([16, bs, C], BF16, tag="sbar")
            for b in range(bs):
                z_tot = pb2.tile([16, 1], F32, tag="z_tot")
                nc.vector.reduce_sum(z_tot[:], zgs[b][:], axis=mybir.AxisListType.X)
                rz = pb2.tile([16, 1], F32, tag="rz")
                nc.vector.reciprocal(rz[:], z_tot[:])
                nc.vector.tensor_scalar_mul(sbar[:, b, :], ssbs[b][:], rz[:])
            stT = pb2.tile([128, NCB, 16 * bs], BF16, tag="stT")
            for j in range(NCB):
                st_ps = ps_sm.tile([128, bs, H], BF16, tag="sm")
                for b in range(bs):
                    nc.tensor.transpose(
                        st_ps[:, b, :],
                        sbar[:, b, j * 128 : (j + 1) * 128],
                        ident[:16, :16],
                    )
                nc.vector.tensor_copy(
                    stT[:, j, :], st_ps.rearrange("p b h -> p (b h)")
                )
            # o_fullT[cr, (b,h)] = sum_c Wv[c,cr] sbar[(b,h),c]; keep h == cr//HD
            o_flatT = pb2.tile([128, 2, bs], BF16, tag="o_flatT")
            for half in range(2):
                of_ps = ps_sm.tile([128, bs, H], F32, tag="sm")
                for j in range(NCB):
                    nc.tensor.matmul(
                        of_ps.rearrange("p b h -> p (b h)"),
                        wv_sb[:, j, half * 128 : (half + 1) * 128],
                        stT[:, j, :],
                        start=(j == 0), stop=(j == NCB - 1),
                    )
                om = pb2.tile([128, bs, H], F32, tag="om")
                nc.vector.tensor_mul(om[:], of_ps[:], dmask_sb[:, half, :, :])
                of_f = pb2.tile([128, bs], F32, tag="of_f")
                nc.vector.reduce_sum(of_f[:], om[:], axis=mybir.AxisListType.X)
                nc.vector.tensor_copy(o_flatT[:, half, :], of_f[:])
            # out[b,:] = o_flat[b] @ Wp + bp
            out_sb = pb2.tile([bs, C], F32, tag="out_sb")
            for half in range(2):
                op_ps = ps_lg.tile([bs, 512], F32, tag="lg", name="op_ps")
                for j in range(2):
                    nc.tensor.matmul(
                        op_ps[:], o_flatT[:, j, :],
                        wp_sb[:, j, half * 512 : (half + 1) * 512],
                        start=(j == 0), stop=(j == 1),
                    )
                nc.vector.tensor_add(
                    out_sb[:, half * 512 : (half + 1) * 512],
                    op_ps[:],
                    bp_sb[:, half * 512 : (half + 1) * 512],
                )
            nc.sync.dma_start(out=out_d, in_=out_sb[:])

        # ---- software-pipelined emission ----
        # Slot k: wtr(k-1), 8 transpose quads of k interleaved with the 8
        # s-matmuls of k-2, then C1(k-1)+exp. PRO(b+1) spreads over g==6/7.
        emit_xq(0)
        for k in range(2):
            emit_dma(k)
        emit_q(0)
        emit_wqeff(0)
        for k in range(nslots):
            b, g = divmod(k, ngroups)
            if g == 0:
                sps[b] = ps_s.tile([16, C], F32, tag="s", name="s_ps")
                if b + 1 < bs:
                    emit_xq(b + 1)
            if k + 2 < nslots:
                emit_dma(k + 2)
            if k >= 2:
                emit_wtr(k - 2)
            for quad in range(8):
                emit_T_quad(k, quad)
                if quad in (1, 2) and k >= 2:
                    emit_C2_pair(k - 2, quad - 1)
                if quad == 3 and k >= 1:
                    emit_C1(k - 1)
                    xts.pop(k - 1)
                if quad in (4, 5) and k >= 2:
                    emit_C2_pair(k - 2, quad - 2)
                if quad == 7 and k >= 2:
                    xgs.pop(k - 2)
                    b2, g2 = divmod(k - 2, ngroups)
                    if g2 == ngroups - 1:
                        emit_sdrain(b2)
            if g == 6 and b + 1 < bs:
                emit_q(b + 1)
            if g == 7 and b + 1 < bs:
                emit_wqeff(b + 1)
        # drain the pipeline tail: wtr(n-2), C2(n-2), C1(n-1)+exp, wtr(n-1),
        # C2(n-1), final s drain, batched epilogue.
        k = nslots
        emit_wtr(k - 2)
        for i in range(4):
            emit_C2_pair(k - 2, i)
        emit_C1(k - 1)
        xts.pop(k - 1)
        emit_wtr(k - 1)
        for i in range(4):
            emit_C2_pair(k - 1, i)
        emit_sdrain(bs - 1)
        emit_E()


def make_hmask():
    hm = np.zeros((128, 2, H), dtype=np.float32)
    for p in range(128):
        for rh in range(2):
            hm[p, rh, (128 * rh + p) // HD] = SCALE * WQ_PRESCALE
    return hm


def make_dmask(bs=BS):
    dm = np.zeros((128, 2, bs, H), dtype=np.float32)
    for p in range(128):
        for half in range(2):
            dm[p, half, :, 8 * half + p // 16] = 1.0
    return dm


def build_bass(bs=BS, n=N):
    nc = bacc.Bacc("TRN2", target_bir_lowering=False, debug=False, num_devices=NCORES)
    x_d = nc.dram_tensor("x", [bs, n, C], F32, kind="ExternalInput").ap()
    wq_d = nc.dram_tensor("Wq", [C, CR], F32, kind="ExternalInput").ap()
    wkt_d = nc.dram_tensor("WkT", [CR, C], F32, kind="ExternalInput").ap()
    wv_d = nc.dram_tensor("Wv", [C, CR], F32, kind="ExternalInput").ap()
    wp_d = nc.dram_tensor("Wp", [CR, C], F32, kind="ExternalInput").ap()
    bp_d = nc.dram_tensor("bp", [BS, C], F32, kind="ExternalInput").ap()
    hmask_d = nc.dram_tensor("hmask", [128, 2, H], F32, kind="ExternalInput").ap()
    dmask_d = nc.dram_tensor("dmask", [128, 2, bs, H], F32, kind="ExternalInput").ap()
    out_d = nc.dram_tensor("out", [bs, C], F32, kind="ExternalOutput").ap()
    with tile.TileContext(nc) as tc:
        emit(tc, x_d, wq_d, wkt_d, wv_d, wp_d, bp_d, hmask_d, dmask_d, out_d, bs, n)
    nc.compile()
    return nc


def make_in_maps(inputs):
    x = np.ascontiguousarray(np.asarray(inputs["x"], dtype=np.float32))
    wq = np.ascontiguousarray(np.asarray(inputs["Wq"], dtype=np.float32))
    wkt = np.ascontiguousarray(np.asarray(inputs["Wk"], dtype=np.float32).T)
    wv = np.ascontiguousarray(np.asarray(inputs["Wv"], dtype=np.float32))
    wp = np.ascontiguousarray(np.asarray(inputs["Wp"], dtype=np.float32))
    bp = np.ascontiguousarray(np.tile(np.asarray(inputs["bp"], dtype=np.float32), (BS, 1)))
    hmask = make_hmask()
    dmask = make_dmask()
    return [
        {
            "x": np.ascontiguousarray(x[c * BS : (c + 1) * BS]),
            "Wq": wq, "WkT": wkt, "Wv": wv, "Wp": wp, "bp": bp,
            "hmask": hmask, "dmask": dmask,
        }
        for c in range(NCORES)
    ]


def run(inputs, trace=False):
    from concourse.bass_utils import run_bass_kernel_spmd

    nc = build_bass()
    in_maps = make_in_maps(inputs)
    res = run_bass_kernel_spmd(
        nc, in_maps, core_ids=list(range(NCORES)), trace=trace
    )
    out = np.concatenate([r["out"] for r in res.results], axis=0)  # [B, C]
    return out.reshape(B, 1, C).astype(np.float32), res


def kernel(**inputs):
    out, _ = run(inputs, trace=False)
    return out


# revision 18
# speedup vs baseline: 1.1347x; 1.1347x over previous
# Trainium2 Bass kernel for ClassAttn (single class-token query attention).
#
# Math (per batch b):
#   q   = x[b,0] @ Wq * scale                       [CR]
#   logits[h,n] = sum_c x[b,n,c] * wq_eff[c,h]      with wq_eff[c,h] = sum_d Wk[c,h*HD+d] q[h*HD+d]
#   w = exp(logits)          (softmax needs no max-subtraction; inputs bounded)
#   z[h] = sum_n w[h,n]
#   s[h,c] = sum_n w[h,n] x[b,n,c]
#   o[h,d] = (1/z[h]) sum_c s[h,c] Wv[c,h*HD+d]
#   out = o.flatten() @ Wp + bp
#
# Sharding: data-parallel over batch, 8 cores x 4 batches, no collectives.
# Per-core the kernel is DMA-paced (64 MiB of x at ~360 GB/s ~ 187 us); the
# PE work per 512-token group is kept under the DMA time:
#   - x transposes: 8 psum quads (4x128x128 each), drains alternate DVE/scalar
#     and cast bf16->fp8 producing xt8.
#   - logits: 4 fp8e4 DoubleRow pairs (2 c-blocks per pass, 0.5 cyc/row);
#     wq_eff is prescaled x32 into fp8-normal range, exp applies scale 1/32.
#   - wq_eff itself is computed on the PE from a host-side transposed Wk
#     (WkT) and a head-masked q-hat (mask folds in SCALE*32), not on DVE.
#   - s-accum stays bf16 from the natural-layout xg.
#   - epilogue (s/z, Wv, Wp, bias) is batched across all 4 batches at the end.

import numpy as np
from contextlib import ExitStack

import concourse.bass as bass
import concourse.mybir as mybir
import concourse.tile as tile
from concourse import bacc
from concourse.masks import make_identity

F32 = mybir.dt.float32
BF16 = mybir.dt.bfloat16
FP8 = mybir.dt.float8e4

B, N, C = 32, 4096, 1024
H, HD = 16, 16
CR = 256
SCALE = HD ** -0.5
NCORES = 8
BS = B // NCORES          # batches per core
GTOK = 512                # tokens per group
BLK = 128                 # tokens per block (partition tile)
NBLK = GTOK // BLK        # 4 blocks per group
NCB = C // 128            # 8 c-blocks
WQ_PRESCALE = 32.0        # wq_eff kept x32 in fp8; exp() applies 1/32


def emit(tc, x_d, wq_d, wkt_d, wv_d, wp_d, bp_d, hmask_d, dmask_d, out_d, bs, n):
    nc = tc.nc
    ngroups = n // GTOK
    nslots = bs * ngroups
    with ExitStack() as ctx:
        const = ctx.enter_context(tc.tile_pool(name="const", bufs=1))
        px = ctx.enter_context(tc.tile_pool(name="px", bufs=5))
        pxt = ctx.enter_context(tc.tile_pool(name="pxt", bufs=3))
        pw = ctx.enter_context(tc.tile_pool(name="pw", bufs=3))
        pb2 = ctx.enter_context(tc.tile_pool(name="pb2", bufs=2))
        pb4 = ctx.enter_context(tc.tile_pool(name="pb4", bufs=4))
        ps_xt = ctx.enter_context(tc.tile_pool(name="ps_xt", bufs=4, space="PSUM"))
        ps_lg = ctx.enter_context(tc.tile_pool(name="ps_lg", bufs=1, space="PSUM"))
        ps_sm = ctx.enter_context(tc.tile_pool(name="ps_sm", bufs=1, space="PSUM"))
        ps_s = ctx.enter_context(tc.tile_pool(name="ps_s", bufs=1, space="PSUM"))

        # ---- constants / weights ----
        first_xgs = []
        for _k0 in range(2):
            _b0, _g0 = divmod(_k0, ngroups)
            xg0 = px.tile([128, NBLK, C], BF16, tag="xg", name="xg0")
            nc.gpsimd.dma_start(
                out=xg0[:],
                in_=x_d[_b0, _g0 * GTOK : (_g0 + 1) * GTOK, :].rearrange(
                    "(p t) c -> p t c", t=NBLK
                ),
            )
            first_xgs.append(xg0)
        ident = const.tile([128, 128], BF16)
        make_identity(nc, ident[:])
        s_inv = const.tile([16, 1], F32)
        nc.vector.memset(s_inv[:], 1.0 / WQ_PRESCALE)

        wq_sb = const.tile([128, NCB, CR], BF16)     # Wq[c,r] c-blocked
        nc.gpsimd.dma_start(out=wq_sb[:], in_=wq_d.rearrange("(p j) r -> p j r", p=128))
        wkt_sb = const.tile([128, 2, C], BF16)       # WkT[r,c] r-blocked
        nc.gpsimd.dma_start(out=wkt_sb[:], in_=wkt_d.rearrange("(j p) c -> p j c", p=128))
        wv_sb = const.tile([128, NCB, CR], BF16)     # Wv[c,r]
        nc.gpsimd.dma_start(out=wv_sb[:], in_=wv_d.rearrange("(j p) r -> p j r", p=128))
        wp_sb = const.tile([128, 2, C], BF16)        # Wp[r,c] r-blocked
        nc.gpsimd.dma_start(out=wp_sb[:], in_=wp_d.rearrange("(j p) c -> p j c", p=128))
        bp_sb = const.tile([bs, C], F32)
        nc.sync.dma_start(out=bp_sb[:], in_=bp_d)
        hmask_sb = const.tile([128, 2, H], F32)      # SCALE*32 at (r, head(r))
        nc.sync.dma_start(out=hmask_sb[:], in_=hmask_d)
        dmask_sb = const.tile([128, 2, bs, H], F32)  # head-extract mask, b-replicated
        nc.sync.dma_start(out=dmask_sb[:], in_=dmask_d)

        xgs = {}      # slot -> xg tile
        xts = {}      # slot -> xt8 tile
        wTs = {}      # slot -> wT tile (exp output)
        wsbs = {}     # slot -> w_sb tile (w natural)
        xqs = {}      # batch -> xq tile
        qhs = {}      # batch -> qh_bf tile
        wq8s = {}     # batch -> wq8 tile
        zgs = {}      # batch -> zg tile
        ssbs = {}     # batch -> s_sb tile
        sps = {}      # batch -> s psum tile

        def emit_xq(b):
            """Class-token channels, c on partitions (tiny strided cast DMA)."""
            xq = px.tile([128, NCB], BF16, tag="xq")
            nc.gpsimd.dma_start(out=xq[:], in_=x_d[b, 0, :].rearrange("(p j) -> p j", p=128))
            xqs[b] = xq

        def emit_q(b):
            """qT[r] = sum_c Wq[c,r] x0[c]; 16 tiny matmuls, rows=1."""
            xq = xqs[b]
            q_ps = ps_sm.tile([128, 2], F32, tag="sm")
            for rh in range(2):
                for j in range(NCB):
                    nc.tensor.matmul(
                        q_ps[:, rh : rh + 1],
                        wq_sb[:, j, rh * 128 : (rh + 1) * 128],
                        xq[:, j : j + 1],
                        start=(j == 0), stop=(j == NCB - 1),
                    )
            q_sb = pb2.tile([128, 2], F32, tag="q_sb")
            nc.vector.tensor_copy(q_sb[:], q_ps[:])
            qh = pb2.tile([128, 2, H], BF16, tag="qh")
            for rh in range(2):
                nc.vector.tensor_scalar_mul(
                    qh[:, rh, :], hmask_sb[:, rh, :], q_sb[:, rh : rh + 1]
                )
            qhs[b] = qh

        def emit_wqeff(b):
            """wq8[c,j,h] = 32*SCALE * sum_d Wk[c,(h,d)] q[(h,d)], fp8."""
            qh = qhs[b]
            wq8 = pb2.tile([128, 4, H], FP8, tag="wq8")
            wqb = pb2.tile([128, 4, H], BF16, tag="wqb")
            for j in range(NCB):
                we_ps = ps_sm.tile([128, H], F32, tag="sm")
                for rh in range(2):
                    nc.tensor.matmul(
                        we_ps[:],
                        wkt_sb[:, rh, j * 128 : (j + 1) * 128],
                        qh[:, rh, :],
                        start=(rh == 0), stop=(rh == 1),
                    )
                if j < 4:
                    nc.vector.tensor_copy(wq8[:, j, :], we_ps[:])
                else:
                    nc.vector.tensor_copy(wqb[:, j - 4, :], we_ps[:])
            wq8s[b] = (wq8, wqb)

        def emit_dma(k):
            if k < 2:
                xgs[k] = first_xgs[k]
                return
            b, g = divmod(k, ngroups)
            xg = px.tile([128, NBLK, C], BF16, tag="xg")
            # Token n' = t*128+p of this group holds DRAM token g*GTOK + 4p + t:
            # 16 KB contiguous DRAM per partition descriptor.
            nc.gpsimd.dma_start(
                out=xg[:],
                in_=x_d[b, g * GTOK : (g + 1) * GTOK, :].rearrange(
                    "(p t) c -> p t c", t=NBLK
                ),
            )
            xgs[k] = xg

        def emit_T_quad(k, quad):
            """4 PE transposes into one psum bank + one casting drain -> xt8."""
            xg = xgs[k]
            if quad == 0:
                xt8_t = pxt.tile([128, 4, GTOK], FP8, tag="xt8", name="xt8")
                xtb_t = pxt.tile([128, 4, GTOK], BF16, tag="xtb", name="xtb")
                xts[k] = (xt8_t, xtb_t)
            xt8, xtb = xts[k]
            blk, jh = divmod(quad, 2)
            xt_ps = ps_xt.tile([128, 4, 128], BF16, tag="xt_ps")
            for jj in range(4):
                j = jh * 4 + jj
                nc.tensor.transpose(
                    xt_ps[:, jj, :], xg[:, blk, j * 128 : (j + 1) * 128], ident[:]
                )
            if jh == 0:
                nc.vector.tensor_copy(
                    xt8[:, :, blk * BLK : (blk + 1) * BLK], xt_ps[:]
                )
            else:
                nc.scalar.copy(
                    xtb[:, :, blk * BLK : (blk + 1) * BLK], xt_ps[:]
                )

        def emit_C2_pair(k, idx):
            """2 of the 8 s-accum matmuls for slot k (bf16, natural xg)."""
            b, g = divmod(k, ngroups)
            xg = xgs[k]
            w_sb = wsbs[k]
            s_ps = sps[b]
            for u in range(2):
                blk, half = divmod(2 * idx + u, 2)
                first = g == 0 and blk == 0
                last = g == ngroups - 1 and blk == NBLK - 1
                nc.tensor.matmul(
                    s_ps[:, half * 512 : (half + 1) * 512],
                    w_sb[:, blk, :],
                    xg[:, blk, half * 512 : (half + 1) * 512],
                    start=first, stop=last,
                )

        def emit_C1(k):
            """logits -> exp(x/32) with z-accum; 4 fp8 DoubleRow pairs."""
            b, g = divmod(k, ngroups)
            xt8, xtb = xts[k]
            wq8, wqb = wq8s[b]
            if g == 0:
                zgs[b] = pb4.tile([16, ngroups], F32, tag="zg", name="zg")
            lg_ps = ps_lg.tile([16, GTOK], F32, tag="lg")
            for p in range(2):
                nc.tensor.matmul(
                    lg_ps[:], wq8[:, 2 * p : 2 * p + 2, :], xt8[:, 2 * p : 2 * p + 2, :],
                    start=(p == 0), stop=False,
                    perf_mode=mybir.MatmulPerfMode.DoubleRow,
                )
            for j in range(4):
                nc.tensor.matmul(
                    lg_ps[:], wqb[:, j, :], xtb[:, j, :],
                    start=False, stop=(j == 3),
                )
            wT = pw.tile([16, GTOK], BF16, tag="wT")
            nc.scalar.activation(
                wT[:], lg_ps[:], mybir.ActivationFunctionType.Exp,
                scale=s_inv[:], accum_out=zgs[b][:, g : g + 1],
            )
            wTs[k] = wT

        def emit_wtr(k):
            """w natural layout via 4 mini PE transposes."""
            wT = wTs.pop(k)
            w_sb = pw.tile([128, NBLK, H], BF16, tag="w_sb")
            w_ps = ps_sm.tile([128, NBLK, H], BF16, tag="sm")
            for blk in range(NBLK):
                nc.tensor.transpose(
                    w_ps[:, blk, :], wT[:, blk * BLK : (blk + 1) * BLK], ident[:16, :16]
                )
            nc.scalar.copy(w_sb[:], w_ps[:])
            wsbs[k] = w_sb

        def emit_sdrain(b):
            """Drain the finished s psum for batch b to SBUF."""
            s_sb = pb4.tile([16, C], F32, tag="s_sb")
            nc.scalar.copy(s_sb[:], sps.pop(b)[:])
            ssbs[b] = s_sb

        def emit_E():
            """Batched epilogue: s/z -> Wv (block-diag) -> Wp + bias, all b."""
            sbar = pb2.tile([16, bs, C], BF16, tag="sbar")
            for b in range(bs):
                z_tot = pb2.tile([16, 1], F32, tag="z_tot")
                nc.vector.reduce_sum(z_tot[:], zgs[b][:], axis=mybir.AxisListType.X)
                rz = pb2.tile([16, 1], F32, tag="rz")
                nc.vector.reciprocal(rz[:], z_tot[:])
                nc.vector.tensor_scalar_mul(sbar[:, b, :], ssbs[b][:], rz[:])
            stT = pb2.tile([128, NCB, 16 * bs], BF16, tag="stT")
            for j in range(NCB):
                st_ps = ps_sm.tile([128, bs, H], BF16, tag="sm")
                for b in range(bs):
                    nc.tensor.transpose(
                        st_ps[:, b, :],
                        sbar[:, b, j * 128 : (j + 1) * 128],
                        ident[:16, :16],
                    )
                nc.vector.tensor_copy(
                    stT[:, j, :], st_ps.rearrange("p b h -> p (b h)")
                )
            # o_fullT[cr, (b,h)] = sum_c Wv[c,cr] sbar[(b,h),c]; keep h == cr//HD
            o_flatT = pb2.tile([128, 2, bs], BF16, tag="o_flatT")
            for half in range(2):
                of_ps = ps_sm.tile([128, bs, H], F32, tag="sm")
                for j in range(NCB):
                    nc.tensor.matmul(
                        of_ps.rearrange("p b h -> p (b h)"),
                        wv_sb[:, j, half * 128 : (half + 1) * 128],
                        stT[:, j, :],
                        start=(j == 0), stop=(j == NCB - 1),
                    )
                om = pb2.tile([128, bs, H], F32, tag="om")
                nc.vector.tensor_mul(om[:], of_ps[:], dmask_sb[:, half, :, :])
                of_f = pb2.tile([128, bs], F32, tag="of_f")
                nc.vector.reduce_sum(of_f[:], om[:], axis=mybir.AxisListType.X)
                nc.vector.tensor_copy(o_flatT[:, half, :], of_f[:])
            # out[b,:] = o_flat[b] @ Wp + bp
            out_sb = pb2.tile([bs, C], F32, tag="out_sb")
            for half in range(2):
                op_ps = ps_lg.tile([bs, 512], F32, tag="lg", name="op_ps")
                for j in range(2):
                    nc.tensor.matmul(
                        op_ps[:], o_flatT[:, j, :],
                        wp_sb[:, j, half * 512 : (half + 1) * 512],
                        start=(j == 0), stop=(j == 1),
                    )
                nc.vector.tensor_add(
                    out_sb[:, half * 512 : (half + 1) * 512],
                    op_ps[:],
                    bp_sb[:, half * 512 : (half + 1) * 512],
                )
            nc.sync.dma_start(out=out_d, in_=out_sb[:])

        # ---- software-pipelined emission ----
        # Slot k: wtr(k-1), 8 transpose quads of k interleaved with the 8
        # s-matmuls of k-2, then C1(k-1)+exp. PRO(b+1) spreads over g==6/7.
        emit_xq(0)
        for k in range(2):
            emit_dma(k)
        emit_q(0)
        emit_wqeff(0)
        for k in range(nslots):
            b, g = divmod(k, ngroups)
            if g == 0:
                sps[b] = ps_s.tile([16, C], F32, tag="s", name="s_ps")
                if b + 1 < bs:
                    emit_xq(b + 1)
            if k + 2 < nslots:
                emit_dma(k + 2)
            if k >= 2:
                emit_wtr(k - 2)
            for quad in range(8):
                emit_T_quad(k, quad)
                if quad in (1, 2) and k >= 2:
                    emit_C2_pair(k - 2, quad - 1)
                if quad == 3 and k >= 1:
                    emit_C1(k - 1)
                    xts.pop(k - 1)
                if quad in (4, 5) and k >= 2:
                    emit_C2_pair(k - 2, quad - 2)
                if quad == 7 and k >= 2:
                    xgs.pop(k - 2)
                    b2, g2 = divmod(k - 2, ngroups)
                    if g2 == ngroups - 1:
                        emit_sdrain(b2)
            if g == 6 and b + 1 < bs:
                emit_q(b + 1)
            if g == 7 and b + 1 < bs:
                emit_wqeff(b + 1)
        # drain the pipeline tail: wtr(n-2), C2(n-2), C1(n-1)+exp, wtr(n-1),
        # C2(n-1), final s drain, batched epilogue.
        k = nslots
        emit_wtr(k - 2)
        for i in range(4):
            emit_C2_pair(k - 2, i)
        emit_C1(k - 1)
        xts.pop(k - 1)
        emit_wtr(k - 1)
        for i in range(4):
            emit_C2_pair(k - 1, i)
        emit_sdrain(bs - 1)
        emit_E()


def make_hmask():
    hm = np.zeros((128, 2, H), dtype=np.float32)
    for p in range(128):
        for rh in range(2):
            hm[p, rh, (128 * rh + p) // HD] = SCALE * WQ_PRESCALE
    return hm


def make_dmask(bs=BS):
    dm = np.zeros((128, 2, bs, H), dtype=np.float32)
    for p in range(128):
        for half in range(2):
            dm[p, half, :, 8 * half + p // 16] = 1.0
    return dm


def build_bass(bs=BS, n=N):
    nc = bacc.Bacc("TRN2", target_bir_lowering=False, debug=False, num_devices=NCORES)
    x_d = nc.dram_tensor("x", [bs, n, C], F32, kind="ExternalInput").ap()
    wq_d = nc.dram_tensor("Wq", [C, CR], F32, kind="ExternalInput").ap()
    wkt_d = nc.dram_tensor("WkT", [CR, C], F32, kind="ExternalInput").ap()
    wv_d = nc.dram_tensor("Wv", [C, CR], F32, kind="ExternalInput").ap()
    wp_d = nc.dram_tensor("Wp", [CR, C], F32, kind="ExternalInput").ap()
    bp_d = nc.dram_tensor("bp", [BS, C], F32, kind="ExternalInput").ap()
    hmask_d = nc.dram_tensor("hmask", [128, 2, H], F32, kind="ExternalInput").ap()
    dmask_d = nc.dram_tensor("dmask", [128, 2, bs, H], F32, kind="ExternalInput").ap()
    out_d = nc.dram_tensor("out", [bs, C], F32, kind="ExternalOutput").ap()
    with tile.TileContext(nc) as tc:
        emit(tc, x_d, wq_d, wkt_d, wv_d, wp_d, bp_d, hmask_d, dmask_d, out_d, bs, n)
    nc.compile()
    return nc


def make_in_maps(inputs):
    x = np.ascontiguousarray(np.asarray(inputs["x"], dtype=np.float32))
    wq = np.ascontiguousarray(np.asarray(inputs["Wq"], dtype=np.float32))
    wkt = np.ascontiguousarray(np.asarray(inputs["Wk"], dtype=np.float32).T)
    wv = np.ascontiguousarray(np.asarray(inputs["Wv"], dtype=np.float32))
    wp = np.ascontiguousarray(np.asarray(inputs["Wp"], dtype=np.float32))
    bp = np.ascontiguousarray(np.tile(np.asarray(inputs["bp"], dtype=np.float32), (BS, 1)))
    hmask = make_hmask()
    dmask = make_dmask()
    return [
        {
            "x": np.ascontiguousarray(x[c * BS : (c + 1) * BS]),
            "Wq": wq, "WkT": wkt, "Wv": wv, "Wp": wp, "bp": bp,
            "hmask": hmask, "dmask": dmask,
        }
        for c in range(NCORES)
    ]


def run(inputs, trace=False):
    from concourse.bass_utils import run_bass_kernel_spmd

    nc = build_bass()
    in_maps = make_in_maps(inputs)
    res = run_bass_kernel_spmd(
        nc, in_maps, core_ids=list(range(NCORES)), trace=trace
    )
    out = np.concatenate([r["out"] for r in res.results], axis=0)  # [B, C]
    return out.reshape(B, 1, C).astype(np.float32), res


def kernel(**inputs):
    out, _ = run(inputs, trace=False)
    return out


# revision 19
# speedup vs baseline: 1.1575x; 1.0200x over previous
# Trainium2 Bass kernel for ClassAttn (single class-token query attention).
#
# Math (per batch b):
#   q   = x[b,0] @ Wq * scale                       [CR]
#   logits[h,n] = sum_c x[b,n,c] * wq_eff[c,h]      with wq_eff[c,h] = sum_d Wk[c,h*HD+d] q[h*HD+d]
#   w = exp(logits)          (softmax needs no max-subtraction; inputs bounded)
#   z[h] = sum_n w[h,n]
#   s[h,c] = sum_n w[h,n] x[b,n,c]
#   o[h,d] = (1/z[h]) sum_c s[h,c] Wv[c,h*HD+d]
#   out = o.flatten() @ Wp + bp
#
# Sharding: data-parallel over batch, 8 cores x 4 batches, no collectives.
# Per-core the kernel is DMA-paced (64 MiB of x at ~360 GB/s ~ 187 us); the
# PE work per 512-token group is kept under the DMA time:
#   - x transposes: 8 psum quads (4x128x128 each), drains alternate DVE/scalar
#     and cast bf16->fp8 producing xt8.
#   - logits: 4 fp8e4 DoubleRow pairs (2 c-blocks per pass, 0.5 cyc/row);
#     wq_eff is prescaled x32 into fp8-normal range, exp applies scale 1/32.
#   - wq_eff itself is computed on the PE from a host-side transposed Wk
#     (WkT) and a head-masked q-hat (mask folds in SCALE*32), not on DVE.
#   - s-accum stays bf16 from the natural-layout xg.
#   - epilogue (s/z, Wv, Wp, bias) is batched across all 4 batches at the end.

import numpy as np
from contextlib import ExitStack

import concourse.bass as bass
import concourse.mybir as mybir
import concourse.tile as tile
from concourse import bacc
from concourse.masks import make_identity

F32 = mybir.dt.float32
BF16 = mybir.dt.bfloat16
FP8 = mybir.dt.float8e4

B, N, C = 32, 4096, 1024
H, HD = 16, 16
CR = 256
SCALE = HD ** -0.5
NCORES = 8
BS = B // NCORES          # batches per core
GTOK = 512                # tokens per group
BLK = 128                 # tokens per block (partition tile)
NBLK = GTOK // BLK        # 4 blocks per group
NCB = C // 128            # 8 c-blocks
WQ_PRESCALE = 32.0        # wq_eff kept x32 in fp8; exp() applies 1/32


def emit(tc, x_d, wq_d, wkt_d, wv_d, wp_d, bp_d, hmask_d, dmask_d, out_d, bs, n):
    nc = tc.nc
    ngroups = n // GTOK
    nslots = bs * ngroups
    with ExitStack() as ctx:
        const = ctx.enter_context(tc.tile_pool(name="const", bufs=1))
        px = ctx.enter_context(tc.tile_pool(name="px", bufs=5))
        pxt = ctx.enter_context(tc.tile_pool(name="pxt", bufs=3))
        pw = ctx.enter_context(tc.tile_pool(name="pw", bufs=3))
        pb2 = ctx.enter_context(tc.tile_pool(name="pb2", bufs=2))
        pb4 = ctx.enter_context(tc.tile_pool(name="pb4", bufs=4))
        ps_xt = ctx.enter_context(tc.tile_pool(name="ps_xt", bufs=4, space="PSUM"))
        ps_lg = ctx.enter_context(tc.tile_pool(name="ps_lg", bufs=1, space="PSUM"))
        ps_sm = ctx.enter_context(tc.tile_pool(name="ps_sm", bufs=1, space="PSUM"))
        ps_s = ctx.enter_context(tc.tile_pool(name="ps_s", bufs=1, space="PSUM"))

        # ---- constants / weights ----
        first_xgs = []
        for _k0 in range(2):
            _b0, _g0 = divmod(_k0, ngroups)
            xg0 = px.tile([128, NBLK, C], BF16, tag="xg", name="xg0")
            nc.gpsimd.dma_start(
                out=xg0[:],
                in_=x_d[_b0, _g0 * GTOK : (_g0 + 1) * GTOK, :].rearrange(
                    "(p t) c -> p t c", t=NBLK
                ),
            )
            first_xgs.append(xg0)
        ident = const.tile([128, 128], BF16)
        make_identity(nc, ident[:])
        s_inv = const.tile([16, 1], F32)
        nc.vector.memset(s_inv[:], 1.0 / WQ_PRESCALE)

        wq_sb = const.tile([128, NCB, CR], BF16)     # Wq[c,r] c-blocked
        nc.gpsimd.dma_start(out=wq_sb[:], in_=wq_d.rearrange("(p j) r -> p j r", p=128))
        wkt_sb = const.tile([128, 2, C], BF16)       # WkT[r,c] r-blocked
        nc.gpsimd.dma_start(out=wkt_sb[:], in_=wkt_d.rearrange("(j p) c -> p j c", p=128))
        wv_sb = const.tile([128, NCB, CR], BF16)     # Wv[c,r]
        nc.gpsimd.dma_start(out=wv_sb[:], in_=wv_d.rearrange("(j p) r -> p j r", p=128))
        wp_sb = const.tile([128, 2, C], BF16)        # Wp[r,c] r-blocked
        nc.gpsimd.dma_start(out=wp_sb[:], in_=wp_d.rearrange("(j p) c -> p j c", p=128))
        bp_sb = const.tile([bs, C], F32)
        nc.sync.dma_start(out=bp_sb[:], in_=bp_d)
        hmask_sb = const.tile([128, 2, H], F32)      # SCALE*32 at (r, head(r))
        nc.sync.dma_start(out=hmask_sb[:], in_=hmask_d)
        dmask_sb = const.tile([128, 2, bs, H], F32)  # head-extract mask, b-replicated
        nc.sync.dma_start(out=dmask_sb[:], in_=dmask_d)

        xgs = {}      # slot -> xg tile
        xts = {}      # slot -> xt8 tile
        wTs = {}      # slot -> wT tile (exp output)
        wsbs = {}     # slot -> w_sb tile (w natural)
        xqs = {}      # batch -> xq tile
        qhs = {}      # batch -> qh_bf tile
        wq8s = {}     # batch -> wq8 tile
        zgs = {}      # batch -> zg tile
        ssbs = {}     # batch -> s_sb tile
        sps = {}      # batch -> s psum tile

        def emit_xq(b):
            """Class-token channels, c on partitions (tiny strided cast DMA)."""
            xq = px.tile([128, NCB], BF16, tag="xq")
            nc.gpsimd.dma_start(out=xq[:], in_=x_d[b, 0, :].rearrange("(p j) -> p j", p=128))
            xqs[b] = xq

        def emit_q(b):
            """qT[r] = sum_c Wq[c,r] x0[c]; 16 tiny matmuls, rows=1."""
            xq = xqs[b]
            q_ps = ps_sm.tile([128, 2], F32, tag="sm")
            for rh in range(2):
                for j in range(NCB):
                    nc.tensor.matmul(
                        q_ps[:, rh : rh + 1],
                        wq_sb[:, j, rh * 128 : (rh + 1) * 128],
                        xq[:, j : j + 1],
                        start=(j == 0), stop=(j == NCB - 1),
                    )
            q_sb = pb2.tile([128, 2], F32, tag="q_sb")
            nc.vector.tensor_copy(q_sb[:], q_ps[:])
            qh = pb2.tile([128, 2, H], BF16, tag="qh")
            for rh in range(2):
                nc.vector.tensor_scalar_mul(
                    qh[:, rh, :], hmask_sb[:, rh, :], q_sb[:, rh : rh + 1]
                )
            qhs[b] = qh

        def emit_wqeff(b):
            """wq8[c,j,h] = 32*SCALE * sum_d Wk[c,(h,d)] q[(h,d)], fp8."""
            qh = qhs[b]
            wq8 = pb2.tile([128, NCB, H], FP8, tag="wq8")
            for j in range(NCB):
                we_ps = ps_sm.tile([128, H], F32, tag="sm")
                for rh in range(2):
                    nc.tensor.matmul(
                        we_ps[:],
                        wkt_sb[:, rh, j * 128 : (j + 1) * 128],
                        qh[:, rh, :],
                        start=(rh == 0), stop=(rh == 1),
                    )
                nc.vector.tensor_copy(wq8[:, j, :], we_ps[:])
            wq8s[b] = wq8

        def emit_dma(k):
            if k < 2:
                xgs[k] = first_xgs[k]
                return
            b, g = divmod(k, ngroups)
            xg = px.tile([128, NBLK, C], BF16, tag="xg")
            # Token n' = t*128+p of this group holds DRAM token g*GTOK + 4p + t:
            # 16 KB contiguous DRAM per partition descriptor.
            nc.gpsimd.dma_start(
                out=xg[:],
                in_=x_d[b, g * GTOK : (g + 1) * GTOK, :].rearrange(
                    "(p t) c -> p t c", t=NBLK
                ),
            )
            xgs[k] = xg

        def emit_T_quad(k, quad):
            """4 PE transposes into one psum bank + one casting drain -> xt8."""
            xg = xgs[k]
            if quad == 0:
                xts[k] = pxt.tile([128, NCB, GTOK], FP8, tag="xt", name="xt8")
            xt8 = xts[k]
            blk, jh = divmod(quad, 2)
            xt_ps = ps_xt.tile([128, 4, 128], BF16, tag="xt_ps")
            for jj in range(4):
                j = jh * 4 + jj
                nc.tensor.transpose(
                    xt_ps[:, jj, :], xg[:, blk, j * 128 : (j + 1) * 128], ident[:]
                )
            dst = xt8[:, jh * 4 : (jh + 1) * 4, blk * BLK : (blk + 1) * BLK]
            nc.vector.tensor_copy(dst, xt_ps[:])

        def emit_C2_pair(k, idx):
            """2 of the 8 s-accum matmuls for slot k (bf16, natural xg)."""
            b, g = divmod(k, ngroups)
            xg = xgs[k]
            w_sb = wsbs[k]
            s_ps = sps[b]
            for u in range(2):
                blk, half = divmod(2 * idx + u, 2)
                first = g == 0 and blk == 0
                last = g == ngroups - 1 and blk == NBLK - 1
                nc.tensor.matmul(
                    s_ps[:, half * 512 : (half + 1) * 512],
                    w_sb[:, blk, :],
                    xg[:, blk, half * 512 : (half + 1) * 512],
                    start=first, stop=last,
                )

        def emit_C1(k):
            """logits -> exp(x/32) with z-accum; 4 fp8 DoubleRow pairs."""
            b, g = divmod(k, ngroups)
            xt8 = xts[k]
            wq8 = wq8s[b]
            if g == 0:
                zgs[b] = pb4.tile([16, ngroups], F32, tag="zg", name="zg")
            lg_ps = ps_lg.tile([16, GTOK], F32, tag="lg")
            for p in range(4):
                nc.tensor.matmul(
                    lg_ps[:], wq8[:, 2 * p : 2 * p + 2, :], xt8[:, 2 * p : 2 * p + 2, :],
                    start=(p == 0), stop=(p == 3),
                    perf_mode=mybir.MatmulPerfMode.DoubleRow,
                )
            wT = pw.tile([16, GTOK], BF16, tag="wT")
            nc.scalar.activation(
                wT[:], lg_ps[:], mybir.ActivationFunctionType.Exp,
                scale=s_inv[:], accum_out=zgs[b][:, g : g + 1],
            )
            wTs[k] = wT

        def emit_wtr(k):
            """w natural layout via 4 mini PE transposes."""
            wT = wTs.pop(k)
            w_sb = pw.tile([128, NBLK, H], BF16, tag="w_sb")
            w_ps = ps_sm.tile([128, NBLK, H], BF16, tag="sm")
            for blk in range(NBLK):
                nc.tensor.transpose(
                    w_ps[:, blk, :], wT[:, blk * BLK : (blk + 1) * BLK], ident[:16, :16]
                )
            nc.scalar.copy(w_sb[:], w_ps[:])
            wsbs[k] = w_sb

        def emit_sdrain(b):
            """Drain the finished s psum for batch b to SBUF."""
            s_sb = pb4.tile([16, C], F32, tag="s_sb")
            nc.scalar.copy(s_sb[:], sps.pop(b)[:])
            ssbs[b] = s_sb

        def emit_E():
            """Batched epilogue: s/z -> Wv (block-diag) -> Wp + bias, all b."""
            sbar = pb2.tile([16, bs, C], BF16, tag="sbar")
            for b in range(bs):
                z_tot = pb2.tile([16, 1], F32, tag="z_tot")
                nc.vector.reduce_sum(z_tot[:], zgs[b][:], axis=mybir.AxisListType.X)
                rz = pb2.tile([16, 1], F32, tag="rz")
                nc.vector.reciprocal(rz[:], z_tot[:])
                nc.vector.tensor_scalar_mul(sbar[:, b, :], ssbs[b][:], rz[:])
            stT = pb2.tile([128, NCB, 16 * bs], BF16, tag="stT")
            for j in range(NCB):
                st_ps = ps_sm.tile([128, bs, H], BF16, tag="sm")
                for b in range(bs):
                    nc.tensor.transpose(
                        st_ps[:, b, :],
                        sbar[:, b, j * 128 : (j + 1) * 128],
                        ident[:16, :16],
                    )
                nc.vector.tensor_copy(
                    stT[:, j, :], st_ps.rearrange("p b h -> p (b h)")
                )
            # o_fullT[cr, (b,h)] = sum_c Wv[c,cr] sbar[(b,h),c]; keep h == cr//HD
            o_flatT = pb2.tile([128, 2, bs], BF16, tag="o_flatT")
            for half in range(2):
                of_ps = ps_sm.tile([128, bs, H], F32, tag="sm")
                for j in range(NCB):
                    nc.tensor.matmul(
                        of_ps.rearrange("p b h -> p (b h)"),
                        wv_sb[:, j, half * 128 : (half + 1) * 128],
                        stT[:, j, :],
                        start=(j == 0), stop=(j == NCB - 1),
                    )
                om = pb2.tile([128, bs, H], F32, tag="om")
                nc.vector.tensor_mul(om[:], of_ps[:], dmask_sb[:, half, :, :])
                of_f = pb2.tile([128, bs], F32, tag="of_f")
                nc.vector.reduce_sum(of_f[:], om[:], axis=mybir.AxisListType.X)
                nc.vector.tensor_copy(o_flatT[:, half, :], of_f[:])
            # out[b,:] = o_flat[b] @ Wp + bp
            out_sb = pb2.tile([bs, C], F32, tag="out_sb")
            for half in range(2):
                op_ps = ps_lg.tile([bs, 512], F32, tag="lg", name="op_ps")
                for j in range(2):
                    nc.tensor.matmul(
                        op_ps[:], o_flatT[:, j, :],
                        wp_sb[:, j, half * 512 : (half + 1) * 512],
                        start=(j == 0), stop=(j == 1),
                    )
                nc.vector.tensor_add(
                    out_sb[:, half * 512 : (half + 1) * 512],
                    op_ps[:],
                    bp_sb[:, half * 512 : (half + 1) * 512],
                )
            nc.sync.dma_start(out=out_d, in_=out_sb[:])

        # ---- software-pipelined emission ----
        # Slot k: wtr(k-1), 8 transpose quads of k interleaved with the 8
        # s-matmuls of k-2, then C1(k-1)+exp. PRO(b+1) spreads over g==6/7.
        emit_xq(0)
        for k in range(2):
            emit_dma(k)
        emit_q(0)
        emit_wqeff(0)
        for k in range(nslots):
            b, g = divmod(k, ngroups)
            if g == 0:
                sps[b] = ps_s.tile([16, C], F32, tag="s", name="s_ps")
                if b + 1 < bs:
                    emit_xq(b + 1)
            if k + 2 < nslots:
                emit_dma(k + 2)
            if k >= 2:
                emit_wtr(k - 2)
            for quad in range(8):
                emit_T_quad(k, quad)
                if quad in (1, 2) and k >= 2:
                    emit_C2_pair(k - 2, quad - 1)
                if quad == 3 and k >= 1:
                    emit_C1(k - 1)
                    xts.pop(k - 1)
                if quad in (4, 5) and k >= 2:
                    emit_C2_pair(k - 2, quad - 2)
                if quad == 7 and k >= 2:
                    xgs.pop(k - 2)
                    b2, g2 = divmod(k - 2, ngroups)
                    if g2 == ngroups - 1:
                        emit_sdrain(b2)
            if g == 6 and b + 1 < bs:
                emit_q(b + 1)
            if g == 7 and b + 1 < bs:
                emit_wqeff(b + 1)
        # drain the pipeline tail: wtr(n-2), C2(n-2), C1(n-1)+exp, wtr(n-1),
        # C2(n-1), final s drain, batched epilogue.
        k = nslots
        emit_wtr(k - 2)
        for i in range(4):
            emit_C2_pair(k - 2, i)
        emit_C1(k - 1)
        xts.pop(k - 1)
        emit_wtr(k - 1)
        for i in range(4):
            emit_C2_pair(k - 1, i)
        emit_sdrain(bs - 1)
        emit_E()


def make_hmask():
    hm = np.zeros((128, 2, H), dtype=np.float32)
    for p in range(128):
        for rh in range(2):
            hm[p, rh, (128 * rh + p) // HD] = SCALE * WQ_PRESCALE
    return hm


def make_dmask(bs=BS):
    dm = np.zeros((128, 2, bs, H), dtype=np.float32)
    for p in range(128):
        for half in range(2):
            dm[p, half, :, 8 * half + p // 16] = 1.0
    return dm


def build_bass(bs=BS, n=N):
    nc = bacc.Bacc("TRN2", target_bir_lowering=False, debug=False, num_devices=NCORES)
    x_d = nc.dram_tensor("x", [bs, n, C], F32, kind="ExternalInput").ap()
    wq_d = nc.dram_tensor("Wq", [C, CR], F32, kind="ExternalInput").ap()
    wkt_d = nc.dram_tensor("WkT", [CR, C], F32, kind="ExternalInput").ap()
    wv_d = nc.dram_tensor("Wv", [C, CR], F32, kind="ExternalInput").ap()
    wp_d = nc.dram_tensor("Wp", [CR, C], F32, kind="ExternalInput").ap()
    bp_d = nc.dram_tensor("bp", [BS, C], F32, kind="ExternalInput").ap()
    hmask_d = nc.dram_tensor("hmask", [128, 2, H], F32, kind="ExternalInput").ap()
    dmask_d = nc.dram_tensor("dmask", [128, 2, bs, H], F32, kind="ExternalInput").ap()
    out_d = nc.dram_tensor("out", [bs, C], F32, kind="ExternalOutput").ap()
    with tile.TileContext(nc) as tc:
        emit(tc, x_d, wq_d, wkt_d, wv_d, wp_d, bp_d, hmask_d, dmask_d, out_d, bs, n)
    nc.compile()
    return nc


def make_in_maps(inputs):
    x = np.ascontiguousarray(np.asarray(inputs["x"], dtype=np.float32))
    wq = np.ascontiguousarray(np.asarray(inputs["Wq"], dtype=np.float32))
    wkt = np.ascontiguousarray(np.asarray(inputs["Wk"], dtype=np.float32).T)
    wv = np.ascontiguousarray(np.asarray(inputs["Wv"], dtype=np.float32))
    wp = np.ascontiguousarray(np.asarray(inputs["Wp"], dtype=np.float32))
    bp = np.ascontiguousarray(np.tile(np.asarray(inputs["bp"], dtype=np.float32), (BS, 1)))
    hmask = make_hmask()
    dmask = make_dmask()
    return [
        {
            "x": np.ascontiguousarray(x[c * BS : (c + 1) * BS]),
            "Wq": wq, "WkT": wkt, "Wv": wv, "Wp": wp, "bp": bp,
            "hmask": hmask, "dmask": dmask,
        }
        for c in range(NCORES)
    ]


def run(inputs, trace=False):
    from concourse.bass_utils import run_bass_kernel_spmd

    nc = build_bass()
    in_maps = make_in_maps(inputs)
    res = run_bass_kernel_spmd(
        nc, in_maps, core_ids=list(range(NCORES)), trace=trace
    )
    out = np.concatenate([r["out"] for r in res.results], axis=0)  # [B, C]
    return out.reshape(B, 1, C).astype(np.float32), res


def kernel(**inputs):
    out, _ = run(inputs, trace=False)
    return out


# revision 20
# speedup vs baseline: 1.2019x; 1.0384x over previous
# Trainium2 Bass kernel for ClassAttn (single class-token query attention).
#
# Math (per batch b):
#   q   = x[b,0] @ Wq * scale                       [CR]
#   logits[h,n] = sum_c x[b,n,c] * wq_eff[c,h]      with wq_eff[c,h] = sum_d Wk[c,h*HD+d] q[h*HD+d]
#   w = exp(logits)          (softmax needs no max-subtraction; inputs bounded)
#   z[h] = sum_n w[h,n]
#   s[h,c] = sum_n w[h,n] x[b,n,c]
#   o[h,d] = (1/z[h]) sum_c s[h,c] Wv[c,h*HD+d]
#   out = o.flatten() @ Wp + bp
#
# Sharding: data-parallel over batch, 8 cores x 4 batches, no collectives.
# Per-core the kernel is DMA-paced (64 MiB of x at ~360 GB/s ~ 187 us); the
# PE work per 512-token group is kept under the DMA time:
#   - x transposes: 8 psum quads (4x128x128 each), drains alternate DVE/scalar
#     and cast bf16->fp8 producing xt8.
#   - logits: 4 fp8e4 DoubleRow pairs (2 c-blocks per pass, 0.5 cyc/row);
#     wq_eff is prescaled x32 into fp8-normal range, exp applies scale 1/32.
#   - wq_eff itself is computed on the PE from a host-side transposed Wk
#     (WkT) and a head-masked q-hat (mask folds in SCALE*32), not on DVE.
#   - s-accum stays bf16 from the natural-layout xg.
#   - epilogue (s/z, Wv, Wp, bias) is batched across all 4 batches at the end.

import numpy as np
from contextlib import ExitStack

import concourse.bass as bass
import concourse.mybir as mybir
import concourse.tile as tile
from concourse import bacc
from concourse.masks import make_identity

F32 = mybir.dt.float32
BF16 = mybir.dt.bfloat16
FP8 = mybir.dt.float8e4

B, N, C = 32, 4096, 1024
H, HD = 16, 16
CR = 256
SCALE = HD ** -0.5
NCORES = 8
BS = B // NCORES          # batches per core
GTOK = 512                # tokens per group
BLK = 128                 # tokens per block (partition tile)
NBLK = GTOK // BLK        # 4 blocks per group
NCB = C // 128            # 8 c-blocks
WQ_PRESCALE = 32.0        # wq_eff kept x32 in fp8; exp() applies 1/32


def emit(tc, x_d, wq_d, wkt_d, wv_d, wp_d, bp_d, hmask_d, dmask_d, out_d, bs, n):
    nc = tc.nc
    ngroups = n // GTOK
    nslots = bs * ngroups
    with ExitStack() as ctx:
        const = ctx.enter_context(tc.tile_pool(name="const", bufs=1))
        px = ctx.enter_context(tc.tile_pool(name="px", bufs=5))
        pxt = ctx.enter_context(tc.tile_pool(name="pxt", bufs=3))
        pw = ctx.enter_context(tc.tile_pool(name="pw", bufs=3))
        pb2 = ctx.enter_context(tc.tile_pool(name="pb2", bufs=2))
        pb4 = ctx.enter_context(tc.tile_pool(name="pb4", bufs=4))
        ps_xt = ctx.enter_context(tc.tile_pool(name="ps_xt", bufs=4, space="PSUM"))
        ps_lg = ctx.enter_context(tc.tile_pool(name="ps_lg", bufs=1, space="PSUM"))
        ps_sm = ctx.enter_context(tc.tile_pool(name="ps_sm", bufs=1, space="PSUM"))
        ps_s = ctx.enter_context(tc.tile_pool(name="ps_s", bufs=1, space="PSUM"))

        # ---- constants / weights ----
        ident = const.tile([128, 128], BF16)
        make_identity(nc, ident[:])
        s_inv = const.tile([16, 1], F32)
        nc.vector.memset(s_inv[:], 1.0 / WQ_PRESCALE)

        wq_sb = const.tile([128, NCB, CR], BF16)     # Wq[c,r] c-blocked
        nc.gpsimd.dma_start(out=wq_sb[:], in_=wq_d.rearrange("(p j) r -> p j r", p=128))
        wkt_sb = const.tile([128, 2, C], BF16)       # WkT[r,c] r-blocked
        nc.gpsimd.dma_start(out=wkt_sb[:], in_=wkt_d.rearrange("(j p) c -> p j c", p=128))
        wv_sb = const.tile([128, NCB, CR], BF16)     # Wv[c,r]
        nc.gpsimd.dma_start(out=wv_sb[:], in_=wv_d.rearrange("(j p) r -> p j r", p=128))
        wp_sb = const.tile([128, 2, C], BF16)        # Wp[r,c] r-blocked
        nc.gpsimd.dma_start(out=wp_sb[:], in_=wp_d.rearrange("(j p) c -> p j c", p=128))
        bp_sb = const.tile([bs, C], F32)
        nc.sync.dma_start(out=bp_sb[:], in_=bp_d)
        hmask_sb = const.tile([128, 2, H], F32)      # SCALE*32 at (r, head(r))
        nc.sync.dma_start(out=hmask_sb[:], in_=hmask_d)
        dmask_sb = const.tile([128, 2, bs, H], F32)  # head-extract mask, b-replicated
        nc.sync.dma_start(out=dmask_sb[:], in_=dmask_d)

        xgs = {}      # slot -> xg tile
        xts = {}      # slot -> xt8 tile
        wTs = {}      # slot -> wT tile (exp output)
        wsbs = {}     # slot -> w_sb tile (w natural)
        xqs = {}      # batch -> xq tile
        qhs = {}      # batch -> qh_bf tile
        wq8s = {}     # batch -> wq8 tile
        zgs = {}      # batch -> zg tile
        ssbs = {}     # batch -> s_sb tile
        sps = {}      # batch -> s psum tile

        def emit_xq(b):
            """Class-token channels, c on partitions (tiny strided cast DMA)."""
            xq = px.tile([128, NCB], BF16, tag="xq")
            nc.gpsimd.dma_start(out=xq[:], in_=x_d[b, 0, :].rearrange("(p j) -> p j", p=128))
            xqs[b] = xq

        def emit_q(b):
            """qT[r] = sum_c Wq[c,r] x0[c]; 16 tiny matmuls, rows=1."""
            xq = xqs[b]
            q_ps = ps_sm.tile([128, 2], F32, tag="sm")
            for rh in range(2):
                for j in range(NCB):
                    nc.tensor.matmul(
                        q_ps[:, rh : rh + 1],
                        wq_sb[:, j, rh * 128 : (rh + 1) * 128],
                        xq[:, j : j + 1],
                        start=(j == 0), stop=(j == NCB - 1),
                    )
            q_sb = pb2.tile([128, 2], F32, tag="q_sb")
            nc.vector.tensor_copy(q_sb[:], q_ps[:])
            qh = pb2.tile([128, 2, H], BF16, tag="qh")
            for rh in range(2):
                nc.vector.tensor_scalar_mul(
                    qh[:, rh, :], hmask_sb[:, rh, :], q_sb[:, rh : rh + 1]
                )
            qhs[b] = qh

        def emit_wqeff(b):
            """wq8[c,j,h] = 32*SCALE * sum_d Wk[c,(h,d)] q[(h,d)], fp8."""
            qh = qhs[b]
            wq8 = pb2.tile([128, NCB, H], FP8, tag="wq8")
            for j in range(NCB):
                we_ps = ps_sm.tile([128, H], F32, tag="sm")
                for rh in range(2):
                    nc.tensor.matmul(
                        we_ps[:],
                        wkt_sb[:, rh, j * 128 : (j + 1) * 128],
                        qh[:, rh, :],
                        start=(rh == 0), stop=(rh == 1),
                    )
                nc.vector.tensor_copy(wq8[:, j, :], we_ps[:])
            wq8s[b] = wq8

        def emit_dma(k):
            b, g = divmod(k, ngroups)
            xg = px.tile([128, NBLK, C], BF16, tag="xg")
            # Token n' = t*128+p of this group holds DRAM token g*GTOK + 4p + t:
            # 16 KB contiguous DRAM per partition descriptor.
            nc.gpsimd.dma_start(
                out=xg[:],
                in_=x_d[b, g * GTOK : (g + 1) * GTOK, :].rearrange(
                    "(p t) c -> p t c", t=NBLK
                ),
            )
            xgs[k] = xg

        def emit_T_quad(k, quad):
            """4 PE transposes into one psum bank + one casting drain -> xt8."""
            xg = xgs[k]
            if quad == 0:
                xts[k] = pxt.tile([128, NCB, GTOK], FP8, tag="xt", name="xt8")
            xt8 = xts[k]
            blk, jh = divmod(quad, 2)
            xt_ps = ps_xt.tile([128, 4, 128], BF16, tag="xt_ps")
            for jj in range(4):
                j = jh * 4 + jj
                nc.tensor.transpose(
                    xt_ps[:, jj, :], xg[:, blk, j * 128 : (j + 1) * 128], ident[:]
                )
            dst = xt8[:, jh * 4 : (jh + 1) * 4, blk * BLK : (blk + 1) * BLK]
            nc.vector.tensor_copy(dst, xt_ps[:])

        def emit_C2_pair(k, idx):
            """2 of the 8 s-accum matmuls for slot k (bf16, natural xg)."""
            b, g = divmod(k, ngroups)
            xg = xgs[k]
            w_sb = wsbs[k]
            s_ps = sps[b]
            for u in range(2):
                blk, half = divmod(2 * idx + u, 2)
                first = g == 0 and blk == 0
                last = g == ngroups - 1 and blk == NBLK - 1
                nc.tensor.matmul(
                    s_ps[:, half * 512 : (half + 1) * 512],
                    w_sb[:, blk, :],
                    xg[:, blk, half * 512 : (half + 1) * 512],
                    start=first, stop=last,
                )

        def emit_C1(k):
            """logits -> exp(x/32) with z-accum; 4 fp8 DoubleRow pairs."""
            b, g = divmod(k, ngroups)
            xt8 = xts[k]
            wq8 = wq8s[b]
            if g == 0:
                zgs[b] = pb4.tile([16, ngroups], F32, tag="zg", name="zg")
            lg_ps = ps_lg.tile([16, GTOK], F32, tag="lg")
            for p in range(4):
                nc.tensor.matmul(
                    lg_ps[:], wq8[:, 2 * p : 2 * p + 2, :], xt8[:, 2 * p : 2 * p + 2, :],
                    start=(p == 0), stop=(p == 3),
                    perf_mode=mybir.MatmulPerfMode.DoubleRow,
                )
            wT = pw.tile([16, GTOK], BF16, tag="wT")
            nc.scalar.activation(
                wT[:], lg_ps[:], mybir.ActivationFunctionType.Exp,
                scale=s_inv[:], accum_out=zgs[b][:, g : g + 1],
            )
            wTs[k] = wT

        def emit_wtr(k):
            """w natural layout via 4 mini PE transposes."""
            wT = wTs.pop(k)
            w_sb = pw.tile([128, NBLK, H], BF16, tag="w_sb")
            w_ps = ps_sm.tile([128, NBLK, H], BF16, tag="sm")
            for blk in range(NBLK):
                nc.tensor.transpose(
                    w_ps[:, blk, :], wT[:, blk * BLK : (blk + 1) * BLK], ident[:16, :16]
                )
            nc.scalar.copy(w_sb[:], w_ps[:])
            wsbs[k] = w_sb

        def emit_sdrain(b):
            """Drain the finished s psum for batch b to SBUF."""
            s_sb = pb4.tile([16, C], F32, tag="s_sb")
            nc.scalar.copy(s_sb[:], sps.pop(b)[:])
            ssbs[b] = s_sb

        def emit_E():
            """Batched epilogue: s/z -> Wv (block-diag) -> Wp + bias, all b."""
            sbar = pb2.tile([16, bs, C], BF16, tag="sbar")
            for b in range(bs):
                z_tot = pb2.tile([16, 1], F32, tag="z_tot")
                nc.vector.reduce_sum(z_tot[:], zgs[b][:], axis=mybir.AxisListType.X)
                rz = pb2.tile([16, 1], F32, tag="rz")
                nc.vector.reciprocal(rz[:], z_tot[:])
                nc.vector.tensor_scalar_mul(sbar[:, b, :], ssbs[b][:], rz[:])
            stT = pb2.tile([128, NCB, 16 * bs], BF16, tag="stT")
            for j in range(NCB):
                st_ps = ps_sm.tile([128, bs, H], BF16, tag="sm")
                for b in range(bs):
                    nc.tensor.transpose(
                        st_ps[:, b, :],
                        sbar[:, b, j * 128 : (j + 1) * 128],
                        ident[:16, :16],
                    )
                nc.vector.tensor_copy(
                    stT[:, j, :], st_ps.rearrange("p b h -> p (b h)")
                )
            # o_fullT[cr, (b,h)] = sum_c Wv[c,cr] sbar[(b,h),c]; keep h == cr//HD
            o_flatT = pb2.tile([128, 2, bs], BF16, tag="o_flatT")
            for half in range(2):
                of_ps = ps_sm.tile([128, bs, H], F32, tag="sm")
                for j in range(NCB):
                    nc.tensor.matmul(
                        of_ps.rearrange("p b h -> p (b h)"),
                        wv_sb[:, j, half * 128 : (half + 1) * 128],
                        stT[:, j, :],
                        start=(j == 0), stop=(j == NCB - 1),
                    )
                om = pb2.tile([128, bs, H], F32, tag="om")
                nc.vector.tensor_mul(om[:], of_ps[:], dmask_sb[:, half, :, :])
                of_f = pb2.tile([128, bs], F32, tag="of_f")
                nc.vector.reduce_sum(of_f[:], om[:], axis=mybir.AxisListType.X)
                nc.vector.tensor_copy(o_flatT[:, half, :], of_f[:])
            # out[b,:] = o_flat[b] @ Wp + bp
            out_sb = pb2.tile([bs, C], F32, tag="out_sb")
            for half in range(2):
                op_ps = ps_lg.tile([bs, 512], F32, tag="lg", name="op_ps")
                for j in range(2):
                    nc.tensor.matmul(
                        op_ps[:], o_flatT[:, j, :],
                        wp_sb[:, j, half * 512 : (half + 1) * 512],
                        start=(j == 0), stop=(j == 1),
                    )
                nc.vector.tensor_add(
                    out_sb[:, half * 512 : (half + 1) * 512],
                    op_ps[:],
                    bp_sb[:, half * 512 : (half + 1) * 512],
                )
            nc.sync.dma_start(out=out_d, in_=out_sb[:])

        # ---- software-pipelined emission ----
        # Slot k: wtr(k-1), 8 transpose quads of k interleaved with the 8
        # s-matmuls of k-2, then C1(k-1)+exp. PRO(b+1) spreads over g==6/7.
        emit_xq(0)
        for k in range(2):
            emit_dma(k)
        emit_q(0)
        emit_wqeff(0)
        for k in range(nslots):
            b, g = divmod(k, ngroups)
            if g == 0:
                sps[b] = ps_s.tile([16, C], F32, tag="s", name="s_ps")
                if b + 1 < bs:
                    emit_xq(b + 1)
            if k + 2 < nslots:
                emit_dma(k + 2)
            if k >= 2:
                emit_wtr(k - 2)
            for quad in range(8):
                emit_T_quad(k, quad)
                if quad in (1, 2) and k >= 2:
                    emit_C2_pair(k - 2, quad - 1)
                if quad == 3 and k >= 1:
                    emit_C1(k - 1)
                    xts.pop(k - 1)
                if quad in (4, 5) and k >= 2:
                    emit_C2_pair(k - 2, quad - 2)
                if quad == 7 and k >= 2:
                    xgs.pop(k - 2)
                    b2, g2 = divmod(k - 2, ngroups)
                    if g2 == ngroups - 1:
                        emit_sdrain(b2)
            if g == 6 and b + 1 < bs:
                emit_q(b + 1)
            if g == 7 and b + 1 < bs:
                emit_wqeff(b + 1)
        # drain the pipeline tail: wtr(n-2), C2(n-2), C1(n-1)+exp, wtr(n-1),
        # C2(n-1), final s drain, batched epilogue.
        k = nslots
        emit_wtr(k - 2)
        for i in range(4):
            emit_C2_pair(k - 2, i)
        emit_C1(k - 1)
        xts.pop(k - 1)
        emit_wtr(k - 1)
        for i in range(4):
            emit_C2_pair(k - 1, i)
        emit_sdrain(bs - 1)
        emit_E()


def make_hmask():
    hm = np.zeros((128, 2, H), dtype=np.float32)
    for p in range(128):
        for rh in range(2):
            hm[p, rh, (128 * rh + p) // HD] = SCALE * WQ_PRESCALE
    return hm


def make_dmask(bs=BS):
    dm = np.zeros((128, 2, bs, H), dtype=np.float32)
    for p in range(128):
        for half in range(2):
            dm[p, half, :, 8 * half + p // 16] = 1.0
    return dm


def build_bass(bs=BS, n=N):
    nc = bacc.Bacc("TRN2", target_bir_lowering=False, debug=False, num_devices=NCORES)
    x_d = nc.dram_tensor("x", [bs, n, C], F32, kind="ExternalInput").ap()
    wq_d = nc.dram_tensor("Wq", [C, CR], F32, kind="ExternalInput").ap()
    wkt_d = nc.dram_tensor("WkT", [CR, C], F32, kind="ExternalInput").ap()
    wv_d = nc.dram_tensor("Wv", [C, CR], F32, kind="ExternalInput").ap()
    wp_d = nc.dram_tensor("Wp", [CR, C], F32, kind="ExternalInput").ap()
    bp_d = nc.dram_tensor("bp", [BS, C], F32, kind="ExternalInput").ap()
    hmask_d = nc.dram_tensor("hmask", [128, 2, H], F32, kind="ExternalInput").ap()
    dmask_d = nc.dram_tensor("dmask", [128, 2, bs, H], F32, kind="ExternalInput").ap()
    out_d = nc.dram_tensor("out", [bs, C], F32, kind="ExternalOutput").ap()
    with tile.TileContext(nc) as tc:
        emit(tc, x_d, wq_d, wkt_d, wv_d, wp_d, bp_d, hmask_d, dmask_d, out_d, bs, n)
    nc.compile()
    return nc


def make_in_maps(inputs):
    x = np.ascontiguousarray(np.asarray(inputs["x"], dtype=np.float32))
    wq = np.ascontiguousarray(np.asarray(inputs["Wq"], dtype=np.float32))
    wkt = np.ascontiguousarray(np.asarray(inputs["Wk"], dtype=np.float32).T)
    wv = np.ascontiguousarray(np.asarray(inputs["Wv"], dtype=np.float32))
    wp = np.ascontiguousarray(np.asarray(inputs["Wp"], dtype=np.float32))
    bp = np.ascontiguousarray(np.tile(np.asarray(inputs["bp"], dtype=np.float32), (BS, 1)))
    hmask = make_hmask()
    dmask = make_dmask()
    return [
        {
            "x": np.ascontiguousarray(x[c * BS : (c + 1) * BS]),
            "Wq": wq, "WkT": wkt, "Wv": wv, "Wp": wp, "bp": bp,
            "hmask": hmask, "dmask": dmask,
        }
        for c in range(NCORES)
    ]


def run(inputs, trace=False):
    from concourse.bass_utils import run_bass_kernel_spmd

    nc = build_bass()
    in_maps = make_in_maps(inputs)
    res = run_bass_kernel_spmd(
        nc, in_maps, core_ids=list(range(NCORES)), trace=trace
    )
    out = np.concatenate([r["out"] for r in res.results], axis=0)  # [B, C]
    return out.reshape(B, 1, C).astype(np.float32), res


def kernel(**inputs):
    out, _ = run(inputs, trace=False)
    return out


# revision 23
# speedup vs baseline: 1.2039x; 1.0017x over previous
# Trainium2 Bass kernel for ClassAttn (single class-token query attention).
#
# Math (per batch b):
#   q   = x[b,0] @ Wq * scale                       [CR]
#   logits[h,n] = sum_c x[b,n,c] * wq_eff[c,h]      with wq_eff[c,h] = sum_d Wk[c,h*HD+d] q[h*HD+d]
#   w = exp(logits)          (softmax needs no max-subtraction; inputs bounded)
#   z[h] = sum_n w[h,n]
#   s[h,c] = sum_n w[h,n] x[b,n,c]
#   o[h,d] = (1/z[h]) sum_c s[h,c] Wv[c,h*HD+d]
#   out = o.flatten() @ Wp + bp
#
# Sharding: data-parallel over batch, 8 cores x 4 batches, no collectives.
# Per-core the kernel is DMA-paced (64 MiB of x at ~360 GB/s ~ 187 us); the
# PE work per 512-token group is kept under the DMA time:
#   - x transposes: 8 psum quads (4x128x128 each), drains alternate DVE/scalar
#     and cast bf16->fp8 producing xt8.
#   - logits: 4 fp8e4 DoubleRow pairs (2 c-blocks per pass, 0.5 cyc/row);
#     wq_eff is prescaled x32 into fp8-normal range, exp applies scale 1/32.
#   - wq_eff itself is computed on the PE from a host-side transposed Wk
#     (WkT) and a head-masked q-hat (mask folds in SCALE*32), not on DVE.
#   - s-accum stays bf16 from the natural-layout xg.
#   - epilogue (s/z, Wv, Wp, bias) is batched across all 4 batches at the end.

import numpy as np
from contextlib import ExitStack

import concourse.bass as bass
import concourse.mybir as mybir
import concourse.tile as tile
from concourse import bacc
from concourse.masks import make_identity

F32 = mybir.dt.float32
BF16 = mybir.dt.bfloat16
FP8 = mybir.dt.float8e4

B, N, C = 32, 4096, 1024
H, HD = 16, 16
CR = 256
SCALE = HD ** -0.5
NCORES = 8
BS = B // NCORES          # batches per core
GTOK = 512                # tokens per group
BLK = 128                 # tokens per block (partition tile)
NBLK = GTOK // BLK        # 4 blocks per group
NCB = C // 128            # 8 c-blocks
WQ_PRESCALE = 32.0        # wq_eff kept x32 in fp8; exp() applies 1/32


def emit(tc, x_d, wq_d, wkt_d, wv_d, wp_d, bp_d, hmask_d, dmask_d, out_d, bs, n):
    nc = tc.nc
    ngroups = n // GTOK
    nslots = bs * ngroups
    with ExitStack() as ctx:
        const = ctx.enter_context(tc.tile_pool(name="const", bufs=1))
        px = ctx.enter_context(tc.tile_pool(name="px", bufs=6))
        pxt = ctx.enter_context(tc.tile_pool(name="pxt", bufs=4))
        pw = ctx.enter_context(tc.tile_pool(name="pw", bufs=3))
        pb2 = ctx.enter_context(tc.tile_pool(name="pb2", bufs=2))
        pb4 = ctx.enter_context(tc.tile_pool(name="pb4", bufs=4))
        ps_xt = ctx.enter_context(tc.tile_pool(name="ps_xt", bufs=4, space="PSUM"))
        ps_lg = ctx.enter_context(tc.tile_pool(name="ps_lg", bufs=1, space="PSUM"))
        ps_sm = ctx.enter_context(tc.tile_pool(name="ps_sm", bufs=1, space="PSUM"))
        ps_s = ctx.enter_context(tc.tile_pool(name="ps_s", bufs=1, space="PSUM"))

        # ---- constants / weights ----
        ident = const.tile([128, 128], BF16)
        make_identity(nc, ident[:])
        s_inv = const.tile([16, 1], F32)
        nc.vector.memset(s_inv[:], 1.0 / WQ_PRESCALE)

        wq_sb = const.tile([128, NCB, CR], BF16)     # Wq[c,r] c-blocked
        nc.gpsimd.dma_start(out=wq_sb[:], in_=wq_d.rearrange("(p j) r -> p j r", p=128))
        wkt_sb = const.tile([128, 2, C], BF16)       # WkT[r,c] r-blocked
        nc.gpsimd.dma_start(out=wkt_sb[:], in_=wkt_d.rearrange("(j p) c -> p j c", p=128))
        wv_sb = const.tile([128, NCB, CR], BF16)     # Wv[c,r]
        nc.gpsimd.dma_start(out=wv_sb[:], in_=wv_d.rearrange("(j p) r -> p j r", p=128))
        wp_sb = const.tile([128, 2, C], BF16)        # Wp[r,c] r-blocked
        nc.gpsimd.dma_start(out=wp_sb[:], in_=wp_d.rearrange("(j p) c -> p j c", p=128))
        bp_sb = const.tile([bs, C], F32)
        nc.sync.dma_start(out=bp_sb[:], in_=bp_d)
        hmask_sb = const.tile([128, 2, H], F32)      # SCALE*32 at (r, head(r))
        nc.sync.dma_start(out=hmask_sb[:], in_=hmask_d)
        dmask_sb = const.tile([128, 2, bs, H], F32)  # head-extract mask, b-replicated
        nc.sync.dma_start(out=dmask_sb[:], in_=dmask_d)

        xgs = {}      # slot -> xg tile
        xts = {}      # slot -> xt8 tile
        wTs = {}      # slot -> wT tile (exp output)
        wsbs = {}     # slot -> w_sb tile (w natural)
        xqs = {}      # batch -> xq tile
        qhs = {}      # batch -> qh_bf tile
        wq8s = {}     # batch -> wq8 tile
        zgs = {}      # batch -> zg tile
        ssbs = {}     # batch -> s_sb tile
        sps = {}      # batch -> s psum tile

        def emit_xq(b):
            """Class-token channels, c on partitions (tiny strided cast DMA)."""
            xq = px.tile([128, NCB], BF16, tag="xq")
            nc.gpsimd.dma_start(out=xq[:], in_=x_d[b, 0, :].rearrange("(p j) -> p j", p=128))
            xqs[b] = xq

        def emit_q(b):
            """qT[r] = sum_c Wq[c,r] x0[c]; 16 tiny matmuls, rows=1."""
            xq = xqs[b]
            q_ps = ps_sm.tile([128, 2], F32, tag="sm")
            for rh in range(2):
                for j in range(NCB):
                    nc.tensor.matmul(
                        q_ps[:, rh : rh + 1],
                        wq_sb[:, j, rh * 128 : (rh + 1) * 128],
                        xq[:, j : j + 1],
                        start=(j == 0), stop=(j == NCB - 1),
                    )
            q_sb = pb2.tile([128, 2], F32, tag="q_sb")
            nc.vector.tensor_copy(q_sb[:], q_ps[:])
            qh = pb2.tile([128, 2, H], BF16, tag="qh")
            for rh in range(2):
                nc.vector.tensor_scalar_mul(
                    qh[:, rh, :], hmask_sb[:, rh, :], q_sb[:, rh : rh + 1]
                )
            qhs[b] = qh

        def emit_wqeff(b):
            """wq8[c,j,h] = 32*SCALE * sum_d Wk[c,(h,d)] q[(h,d)], fp8."""
            qh = qhs[b]
            wq8 = pb2.tile([128, NCB, H], FP8, tag="wq8")
            for j in range(NCB):
                we_ps = ps_sm.tile([128, H], F32, tag="sm")
                for rh in range(2):
                    nc.tensor.matmul(
                        we_ps[:],
                        wkt_sb[:, rh, j * 128 : (j + 1) * 128],
                        qh[:, rh, :],
                        start=(rh == 0), stop=(rh == 1),
                    )
                nc.vector.tensor_copy(wq8[:, j, :], we_ps[:])
            wq8s[b] = wq8

        def emit_dma(k):
            b, g = divmod(k, ngroups)
            xg = px.tile([128, NBLK, C], BF16, tag="xg")
            # Token n' = t*128+p of this group holds DRAM token g*GTOK + 4p + t:
            # 16 KB contiguous DRAM per partition descriptor.
            nc.gpsimd.dma_start(
                out=xg[:],
                in_=x_d[b, g * GTOK : (g + 1) * GTOK, :].rearrange(
                    "(p t) c -> p t c", t=NBLK
                ),
            )
            xgs[k] = xg

        def emit_T_quad(k, quad):
            """4 PE transposes into one psum bank + one casting drain -> xt8."""
            xg = xgs[k]
            if quad == 0:
                xts[k] = pxt.tile([128, NCB, GTOK], FP8, tag="xt", name="xt8")
            xt8 = xts[k]
            blk, jh = divmod(quad, 2)
            xt_ps = ps_xt.tile([128, 4, 128], BF16, tag="xt_ps")
            for jj in range(4):
                j = jh * 4 + jj
                nc.tensor.transpose(
                    xt_ps[:, jj, :], xg[:, blk, j * 128 : (j + 1) * 128], ident[:]
                )
            dst = xt8[:, jh * 4 : (jh + 1) * 4, blk * BLK : (blk + 1) * BLK]
            nc.vector.tensor_copy(dst, xt_ps[:])

        def emit_C2_pair(k, idx):
            """2 of the 8 s-accum matmuls for slot k (bf16, natural xg)."""
            b, g = divmod(k, ngroups)
            xg = xgs[k]
            w_sb = wsbs[k]
            s_ps = sps[b]
            for u in range(2):
                blk, half = divmod(2 * idx + u, 2)
                first = g == 0 and blk == 0
                last = g == ngroups - 1 and blk == NBLK - 1
                nc.tensor.matmul(
                    s_ps[:, half * 512 : (half + 1) * 512],
                    w_sb[:, blk, :],
                    xg[:, blk, half * 512 : (half + 1) * 512],
                    start=first, stop=last,
                )

        def emit_C1(k):
            """logits -> exp(x/32) with z-accum; 4 fp8 DoubleRow pairs."""
            b, g = divmod(k, ngroups)
            xt8 = xts[k]
            wq8 = wq8s[b]
            if g == 0:
                zgs[b] = pb4.tile([16, ngroups], F32, tag="zg", name="zg")
            lg_ps = ps_lg.tile([16, GTOK], F32, tag="lg")
            for p in range(4):
                nc.tensor.matmul(
                    lg_ps[:], wq8[:, 2 * p : 2 * p + 2, :], xt8[:, 2 * p : 2 * p + 2, :],
                    start=(p == 0), stop=(p == 3),
                    perf_mode=mybir.MatmulPerfMode.DoubleRow,
                )
            wT = pw.tile([16, GTOK], BF16, tag="wT")
            nc.scalar.activation(
                wT[:], lg_ps[:], mybir.ActivationFunctionType.Exp,
                scale=s_inv[:], accum_out=zgs[b][:, g : g + 1],
            )
            wTs[k] = wT

        def emit_wtr(k):
            """w natural layout via 4 mini PE transposes."""
            wT = wTs.pop(k)
            w_sb = pw.tile([128, NBLK, H], BF16, tag="w_sb")
            w_ps = ps_sm.tile([128, NBLK, H], BF16, tag="sm")
            for blk in range(NBLK):
                nc.tensor.transpose(
                    w_ps[:, blk, :], wT[:, blk * BLK : (blk + 1) * BLK], ident[:16, :16]
                )
            nc.scalar.copy(w_sb[:], w_ps[:])
            wsbs[k] = w_sb

        def emit_sdrain(b):
            """Drain the finished s psum for batch b to SBUF."""
            s_sb = pb4.tile([16, C], F32, tag="s_sb")
            nc.scalar.copy(s_sb[:], sps.pop(b)[:])
            ssbs[b] = s_sb

        def emit_E():
            """Batched epilogue: s/z -> Wv (block-diag) -> Wp + bias, all b."""
            sbar = pb2.tile([16, bs, C], BF16, tag="sbar")
            for b in range(bs):
                z_tot = pb2.tile([16, 1], F32, tag="z_tot")
                nc.vector.reduce_sum(z_tot[:], zgs[b][:], axis=mybir.AxisListType.X)
                rz = pb2.tile([16, 1], F32, tag="rz")
                nc.vector.reciprocal(rz[:], z_tot[:])
                nc.vector.tensor_scalar_mul(sbar[:, b, :], ssbs[b][:], rz[:])
            stT = pb2.tile([128, NCB, 16 * bs], BF16, tag="stT")
            for j in range(NCB):
                st_ps = ps_sm.tile([128, bs, H], BF16, tag="sm")
                for b in range(bs):
                    nc.tensor.transpose(
                        st_ps[:, b, :],
                        sbar[:, b, j * 128 : (j + 1) * 128],
                        ident[:16, :16],
                    )
                nc.vector.tensor_copy(
                    stT[:, j, :], st_ps.rearrange("p b h -> p (b h)")
                )
            # o_fullT[cr, (b,h)] = sum_c Wv[c,cr] sbar[(b,h),c]; keep h == cr//HD
            o_flatT = pb2.tile([128, 2, bs], BF16, tag="o_flatT")
            for half in range(2):
                of_ps = ps_sm.tile([128, bs, H], F32, tag="sm")
                for j in range(NCB):
                    nc.tensor.matmul(
                        of_ps.rearrange("p b h -> p (b h)"),
                        wv_sb[:, j, half * 128 : (half + 1) * 128],
                        stT[:, j, :],
                        start=(j == 0), stop=(j == NCB - 1),
                    )
                om = pb2.tile([128, bs, H], F32, tag="om")
                nc.vector.tensor_mul(om[:], of_ps[:], dmask_sb[:, half, :, :])
                of_f = pb2.tile([128, bs], F32, tag="of_f")
                nc.vector.reduce_sum(of_f[:], om[:], axis=mybir.AxisListType.X)
                nc.vector.tensor_copy(o_flatT[:, half, :], of_f[:])
            # out[b,:] = o_flat[b] @ Wp + bp
            out_sb = pb2.tile([bs, C], F32, tag="out_sb")
            for half in range(2):
                op_ps = ps_lg.tile([bs, 512], F32, tag="lg", name="op_ps")
                for j in range(2):
                    nc.tensor.matmul(
                        op_ps[:], o_flatT[:, j, :],
                        wp_sb[:, j, half * 512 : (half + 1) * 512],
                        start=(j == 0), stop=(j == 1),
                    )
                nc.vector.tensor_add(
                    out_sb[:, half * 512 : (half + 1) * 512],
                    op_ps[:],
                    bp_sb[:, half * 512 : (half + 1) * 512],
                )
            nc.sync.dma_start(out=out_d, in_=out_sb[:])

        # ---- software-pipelined emission ----
        # Slot k: wtr(k-1), 8 transpose quads of k interleaved with the 8
        # s-matmuls of k-2, then C1(k-1)+exp. PRO(b+1) spreads over g==6/7.
        emit_xq(0)
        for k in range(2):
            emit_dma(k)
        emit_q(0)
        emit_wqeff(0)
        for k in range(nslots):
            b, g = divmod(k, ngroups)
            if g == 0:
                sps[b] = ps_s.tile([16, C], F32, tag="s", name="s_ps")
                if b + 1 < bs:
                    emit_xq(b + 1)
            if k + 2 < nslots:
                emit_dma(k + 2)
            if k >= 2:
                emit_wtr(k - 2)
            for quad in range(8):
                emit_T_quad(k, quad)
                if quad in (1, 2, 3, 4) and k >= 2:
                    emit_C2_pair(k - 2, quad - 1)
                if quad == 5 and k >= 1:
                    emit_C1(k - 1)
                    xts.pop(k - 1)
                if quad == 7 and k >= 2:
                    xgs.pop(k - 2)
                    b2, g2 = divmod(k - 2, ngroups)
                    if g2 == ngroups - 1:
                        emit_sdrain(b2)
            if g == 6 and b + 1 < bs:
                emit_q(b + 1)
            if g == 7 and b + 1 < bs:
                emit_wqeff(b + 1)
        # drain the pipeline tail: wtr(n-2), C2(n-2), C1(n-1)+exp, wtr(n-1),
        # C2(n-1), final s drain, batched epilogue.
        k = nslots
        emit_wtr(k - 2)
        for i in range(4):
            emit_C2_pair(k - 2, i)
        emit_C1(k - 1)
        xts.pop(k - 1)
        emit_wtr(k - 1)
        for i in range(4):
            emit_C2_pair(k - 1, i)
        emit_sdrain(bs - 1)
        emit_E()


def make_hmask():
    hm = np.zeros((128, 2, H), dtype=np.float32)
    for p in range(128):
        for rh in range(2):
            hm[p, rh, (128 * rh + p) // HD] = SCALE * WQ_PRESCALE
    return hm


def make_dmask(bs=BS):
    dm = np.zeros((128, 2, bs, H), dtype=np.float32)
    for p in range(128):
        for half in range(2):
            dm[p, half, :, 8 * half + p // 16] = 1.0
    return dm


def build_bass(bs=BS, n=N):
    nc = bacc.Bacc("TRN2", target_bir_lowering=False, debug=False, num_devices=NCORES)
    x_d = nc.dram_tensor("x", [bs, n, C], F32, kind="ExternalInput").ap()
    wq_d = nc.dram_tensor("Wq", [C, CR], F32, kind="ExternalInput").ap()
    wkt_d = nc.dram_tensor("WkT", [CR, C], F32, kind="ExternalInput").ap()
    wv_d = nc.dram_tensor("Wv", [C, CR], F32, kind="ExternalInput").ap()
    wp_d = nc.dram_tensor("Wp", [CR, C], F32, kind="ExternalInput").ap()
    bp_d = nc.dram_tensor("bp", [BS, C], F32, kind="ExternalInput").ap()
    hmask_d = nc.dram_tensor("hmask", [128, 2, H], F32, kind="ExternalInput").ap()
    dmask_d = nc.dram_tensor("dmask", [128, 2, bs, H], F32, kind="ExternalInput").ap()
    out_d = nc.dram_tensor("out", [bs, C], F32, kind="ExternalOutput").ap()
    with tile.TileContext(nc) as tc:
        emit(tc, x_d, wq_d, wkt_d, wv_d, wp_d, bp_d, hmask_d, dmask_d, out_d, bs, n)
    nc.compile()
    return nc


def make_in_maps(inputs):
    x = np.ascontiguousarray(np.asarray(inputs["x"], dtype=np.float32))
    wq = np.ascontiguousarray(np.asarray(inputs["Wq"], dtype=np.float32))
    wkt = np.ascontiguousarray(np.asarray(inputs["Wk"], dtype=np.float32).T)
    wv = np.ascontiguousarray(np.asarray(inputs["Wv"], dtype=np.float32))
    wp = np.ascontiguousarray(np.asarray(inputs["Wp"], dtype=np.float32))
    bp = np.ascontiguousarray(np.tile(np.asarray(inputs["bp"], dtype=np.float32), (BS, 1)))
    hmask = make_hmask()
    dmask = make_dmask()
    return [
        {
            "x": np.ascontiguousarray(x[c * BS : (c + 1) * BS]),
            "Wq": wq, "WkT": wkt, "Wv": wv, "Wp": wp, "bp": bp,
            "hmask": hmask, "dmask": dmask,
        }
        for c in range(NCORES)
    ]


def run(inputs, trace=False):
    from concourse.bass_utils import run_bass_kernel_spmd

    nc = build_bass()
    in_maps = make_in_maps(inputs)
    res = run_bass_kernel_spmd(
        nc, in_maps, core_ids=list(range(NCORES)), trace=trace
    )
    out = np.concatenate([r["out"] for r in res.results], axis=0)  # [B, C]
    return out.reshape(B, 1, C).astype(np.float32), res


def kernel(**inputs):
    out, _ = run(inputs, trace=False)
    return out
